# revision 14
# baseline (speedup 1.0000x reference)
"""Grouped MoE (top-2 of 8 experts, SwiGLU) on 8 Trainium2 NeuronCores.

Sharding: expert-parallel with real token dispatch. The top-2 gate is
computed on host (33 MFLOP of numpy, exactly reproducing the reference's
softmax/top-k math); tokens are gathered per expert on host. Core c owns
expert c and runs the three expert GEMMs in bf16 over only the tokens
routed to expert c (padded to a fixed capacity C, a multiple of 128),
scales each output row by the host-computed renormalized gate weight,
and writes its [C, D] partial output. The host scatter-adds the two
expert partials per token into the full [T, D] fp32 output.

No collectives: each (token, expert) pair is computed on exactly one
core, so combining is a disjoint scatter-add on host.

Device kernel: tokens are processed in 384-wide chunks. Per chunk,
GEMM1 runs k-outer/f-inner so the tensor engine consumes inputs in
exactly the order the DMAs deliver them (w1+x first, then w3, then w2),
using all 8 PSUM banks per phase; phases are software-pipelined across
chunks (A0 A1 B0 G0 A2 B1 G1 ...). DMA issues are split across the two
hardware-DGE sequencers (sync, scalar) because each dma_start costs
~0.6us of sequencer issue time.
"""

import sys
import numpy as np

for _p in ("/opt/trn_rl_repo",):
    if _p not in sys.path:
        sys.path.insert(0, _p)

B, S, D, F, E = 2, 2048, 1024, 1024, 8
T = B * S            # 4096 tokens
NCORES = 8
P = 128
DK = D // P          # 8 contraction chunks over D
FK = F // P          # 8 F tiles
TCH = 384            # token chunk (3 PSUM banks of headroom per bank set)

_cache = {}


def _build_nc(C):
    """Expert kernel over C routed tokens (C a multiple of 128)."""
    import itertools
    from contextlib import ExitStack

    import concourse.mybir as mybir
    import concourse.tile as tile
    from concourse import bacc

    dt = mybir.dt
    AF = mybir.ActivationFunctionType
    ALU = mybir.AluOpType

    NTT = C // P
    chunks = []
    t = 0
    while t < C:
        c = min(TCH, C - t)
        chunks.append((t, c))
        t += c
    NCH = len(chunks)

    nc = bacc.Bacc("TRN2", target_bir_lowering=False, debug=False,
                   num_devices=NCORES)

    xg = nc.dram_tensor("xg", [D, C], dt.bfloat16, kind="ExternalInput").ap()
    w1t = nc.dram_tensor("w1t", [D, F], dt.bfloat16, kind="ExternalInput").ap()
    w3t = nc.dram_tensor("w3t", [D, F], dt.bfloat16, kind="ExternalInput").ap()
    w2t = nc.dram_tensor("w2t", [F, D], dt.bfloat16, kind="ExternalInput").ap()
    gs = nc.dram_tensor("gs", [P, NTT], dt.float32, kind="ExternalInput").ap()
    out = nc.dram_tensor("out", [C, D], dt.bfloat16, kind="ExternalOutput").ap()

    with tile.TileContext(nc) as tc, ExitStack() as ctx:
        const = ctx.enter_context(tc.tile_pool(name="const", bufs=1))
        spool = ctx.enter_context(tc.tile_pool(name="spool", bufs=2))
        hpool = ctx.enter_context(tc.tile_pool(name="hpool", bufs=2))
        ypool = ctx.enter_context(tc.tile_pool(name="ypool", bufs=3))
        psum = ctx.enter_context(tc.tile_pool(name="psum", bufs=1, space="PSUM"))

        # ---- input DMAs, balanced across the two HWDGE issue queues ----
        # Priority order = PE consumption order: (w1_k | xgc0_k) pairs,
        # then x chunk1, then w3, then x chunk2+, then w2 halves, then gs.
        # Alternate engines per k so both DMA paths carry ~half the bytes.
        engs = [nc.sync, nc.scalar]
        w1_sb = []
        xgc = [[None] * NCH for _ in range(DK)]
        for k in range(DK):
            t1 = const.tile([P, F], dt.bfloat16, tag=f"w1_{k}")
            engs[k % 2].dma_start(t1[:], w1t[k * P:(k + 1) * P, :])
            w1_sb.append(t1)
            tok, tch = chunks[0]
            tx = const.tile([P, tch], dt.bfloat16, tag=f"xg{k}c0")
            engs[(k + 1) % 2].dma_start(
                tx[:], xg[k * P:(k + 1) * P, tok:tok + tch])
            xgc[k][0] = tx
        for ci in range(1, NCH):
            tok, tch = chunks[ci]
            for k in range(DK):
                tx = const.tile([P, tch], dt.bfloat16, tag=f"xg{k}c{ci}")
                engs[(k + ci) % 2].dma_start(
                    tx[:], xg[k * P:(k + 1) * P, tok:tok + tch])
                xgc[k][ci] = tx
        w3_sb = []
        for k in range(DK):
            t3 = const.tile([P, F], dt.bfloat16, tag=f"w3_{k}")
            engs[k % 2].dma_start(t3[:], w3t[k * P:(k + 1) * P, :])
            w3_sb.append(t3)
        # w2 in two half DMAs (one per engine) into a [P, FK, D] tile;
        # slice fk is w2t rows fk*128..(fk+1)*128
        w2_sb = const.tile([P, FK, D], dt.bfloat16, tag="w2")
        HK = FK // 2
        nc.sync.dma_start(
            w2_sb[:, 0:HK, :],
            w2t[0:HK * P, :].rearrange("(k p) d -> p k d", p=P))
        nc.scalar.dma_start(
            w2_sb[:, HK:FK, :],
            w2t[HK * P:F, :].rearrange("(k p) d -> p k d", p=P))
        gs_sb = const.tile([P, NTT], dt.float32, tag="gs")
        nc.scalar.dma_start(gs_sb[:], gs[:, :])

        slot = itertools.count()

        # PE warmup: a few matmuls on a zeroed dummy tile fill the otherwise
        # idle window while the first real tiles stream in, and trip the HAM
        # activity monitor early so real matmuls run at the full 2.4 GHz.
        dummy = const.tile([P, 512], dt.bfloat16, tag="dummy")
        nc.vector.memset(dummy[:], 0)
        psW = psum.tile([P, TCH], dt.float32, tag=f"b{next(slot) % 8}")
        for _ in range(12):
            nc.tensor.matmul(psW[:], lhsT=dummy[:, 0:P],
                             rhs=dummy[:, P:P + TCH], start=True, stop=True)

        psA = [None] * NCH
        psB = [None] * NCH
        ssb = [None] * NCH
        hsb = [None] * NCH

        def emit_A(ci):
            tok, tch = chunks[ci]
            tiles = []
            for f in range(FK):
                pa = psum.tile([P, tch], dt.float32, tag=f"b{next(slot) % 8}")
                tiles.append(pa)
            psA[ci] = tiles
            for k in range(DK):
                for f in range(FK):
                    nc.tensor.matmul(
                        tiles[f][:], lhsT=w1_sb[k][:, f * P:(f + 1) * P],
                        rhs=xgc[k][ci][:],
                        start=(k == 0), stop=(k == DK - 1))
            # silu into SBUF frees the psA banks for the next phase
            stiles = []
            for f in range(FK):
                st = spool.tile([P, tch], dt.float32, tag=f"s{f}")
                nc.scalar.activation(st[:], tiles[f][:], AF.Silu)
                stiles.append(st)
            ssb[ci] = stiles

        def emit_B(ci):
            tok, tch = chunks[ci]
            tiles = []
            for f in range(FK):
                pb = psum.tile([P, tch], dt.float32, tag=f"b{next(slot) % 8}")
                tiles.append(pb)
            psB[ci] = tiles
            for k in range(DK):
                for f in range(FK):
                    nc.tensor.matmul(
                        tiles[f][:], lhsT=w3_sb[k][:, f * P:(f + 1) * P],
                        rhs=xgc[k][ci][:],
                        start=(k == 0), stop=(k == DK - 1))
            htiles = []
            for f in range(FK):
                ht = hpool.tile([P, tch], dt.bfloat16, tag=f"h{f}")
                nc.vector.tensor_tensor(ht[:], ssb[ci][f][:], tiles[f][:],
                                        op=ALU.mult)
                htiles.append(ht)
            hsb[ci] = htiles

        def emit_G(ci):
            tok, tch = chunks[ci]
            for m in range(tch // P):
                jj = tok // P + m
                ysb = ypool.tile([P, D], dt.bfloat16, tag="ysb")
                psY0 = psum.tile([P, 512], dt.float32, tag=f"b{next(slot) % 8}")
                psY1 = psum.tile([P, 512], dt.float32, tag=f"b{next(slot) % 8}")
                for fk in range(FK):
                    hT = hsb[ci][fk][:, m * P:(m + 1) * P]
                    nc.tensor.matmul(
                        psY0[:], lhsT=hT,
                        rhs=w2_sb[:, fk, 0:512],
                        start=(fk == 0), stop=(fk == FK - 1))
                    nc.tensor.matmul(
                        psY1[:], lhsT=hT,
                        rhs=w2_sb[:, fk, 512:1024],
                        start=(fk == 0), stop=(fk == FK - 1))
                # scale+copy the two halves on different engines (ACT | DVE)
                # and issue their output DMAs from different queues, so the
                # per-tile epilogue runs in parallel rather than serially.
                nc.scalar.activation(ysb[:, 0:512], psY0[:], AF.Copy,
                                     scale=gs_sb[:, jj:jj + 1])
                nc.vector.tensor_scalar_mul(ysb[:, 512:1024], psY1[:],
                                            gs_sb[:, jj:jj + 1])
                nc.sync.dma_start(
                    out[tok + m * P: tok + (m + 1) * P, 0:512],
                    ysb[:, 0:512])
                nc.gpsimd.dma_start(
                    out[tok + m * P: tok + (m + 1) * P, 512:1024],
                    ysb[:, 512:1024])

        # software pipeline: A_{i+1} runs ahead so GEMM1-B never waits on w3
        if NCH >= 2:
            emit_A(0)
            emit_A(1)
        else:
            emit_A(0)
        for i in range(NCH):
            emit_B(i)
            emit_G(i)
            if i + 2 < NCH:
                emit_A(i + 2)

    nc.compile()
    return nc


def _route(x, gate_w):
    """Host gate: top-2 of 8, renormalized weights; per-expert token lists."""
    xf = np.ascontiguousarray(x.reshape(T, D).astype(np.float32))
    logits = xf @ gate_w.T.astype(np.float32)            # [T, E]
    order = np.argsort(-logits, axis=1, kind="stable")
    i1 = order[:, 0]
    i2 = order[:, 1]
    ar = np.arange(T)
    l1 = logits[ar, i1]
    l2 = logits[ar, i2]
    g1 = 1.0 / (1.0 + np.exp(l2 - l1))                   # renormalized top-2
    g2 = 1.0 - g1
    idxs, gws = [], []
    for e in range(E):
        m1 = i1 == e
        m2 = i2 == e
        idx = np.nonzero(m1 | m2)[0]
        g = np.where(m1, g1, g2)[idx].astype(np.float32)
        idxs.append(idx)
        gws.append(g)
    maxn = max(len(i) for i in idxs)
    C = max(512, -(-maxn // P) * P)
    return xf, idxs, gws, C


def _prepare(x, gate_w, w1, w3, w2):
    """Build (nc, in_maps, route_meta) for an SPMD run."""
    import ml_dtypes

    xf, idxs, gws, C = _route(x, gate_w)
    xTb = np.ascontiguousarray(xf.T).astype(ml_dtypes.bfloat16)  # [D, T]

    key = ("nc", C)
    if key not in _cache:
        _cache[key] = _build_nc(C)
    nc = _cache[key]

    in_maps = []
    for c in range(NCORES):
        idx = idxs[c]
        n = len(idx)
        xg = np.zeros((D, C), dtype=ml_dtypes.bfloat16)
        xg[:, :n] = xTb[:, idx]
        gpad = np.zeros(C, dtype=np.float32)
        gpad[:n] = gws[c]
        gs = np.ascontiguousarray(gpad.reshape(C // P, P).T)     # [P, NTT]
        in_maps.append({
            "xg": xg,
            "w1t": np.ascontiguousarray(w1[c].T).astype(ml_dtypes.bfloat16),
            "w3t": np.ascontiguousarray(w3[c].T).astype(ml_dtypes.bfloat16),
            "w2t": np.ascontiguousarray(w2[c].T).astype(ml_dtypes.bfloat16),
            "gs": gs,
        })
    return nc, in_maps, (idxs, C)


def _combine(results, meta):
    idxs, C = meta
    outf = np.zeros((T, D), dtype=np.float32)
    for e in range(E):
        idx = idxs[e]
        y = np.asarray(results[e]["out"])[:len(idx)].astype(np.float32)
        outf[idx] += y
    return outf.reshape(B, S, D)


def kernel(x, gate_w, w1, w3, w2):
    from concourse.bass_utils import run_bass_kernel_spmd

    nc, in_maps, meta = _prepare(x, gate_w, w1, w3, w2)
    res = run_bass_kernel_spmd(nc, in_maps, list(range(NCORES)))
    return _combine(res.results, meta)


# revision 15
# speedup vs baseline: 1.1127x; 1.1127x over previous
"""Grouped MoE (top-2 of 8 experts, SwiGLU) on 8 Trainium2 NeuronCores.

Sharding: expert-parallel with real token dispatch. The top-2 gate is
computed on host (33 MFLOP of numpy, exactly reproducing the reference's
softmax/top-k math); tokens are gathered per expert on host. Core c owns
expert c and runs the three expert GEMMs in bf16 over only the tokens
routed to expert c (padded to a fixed capacity C, a multiple of 128),
scales each output row by the host-computed renormalized gate weight,
and writes its [C, D] partial output. The host scatter-adds the two
expert partials per token into the full [T, D] fp32 output.

No collectives: each (token, expert) pair is computed on exactly one
core, so combining is a disjoint scatter-add on host.

Device kernel: tokens are processed in 384-wide chunks. Per chunk,
GEMM1 runs k-outer/f-inner so the tensor engine consumes inputs in
exactly the order the DMAs deliver them (w1+x first, then w3, then w2),
using all 8 PSUM banks per phase; phases are software-pipelined across
chunks (A0 A1 B0 G0 A2 B1 G1 ...). DMA issues are split across the two
hardware-DGE sequencers (sync, scalar) because each dma_start costs
~0.6us of sequencer issue time.
"""

import sys
import numpy as np

for _p in ("/opt/trn_rl_repo",):
    if _p not in sys.path:
        sys.path.insert(0, _p)

B, S, D, F, E = 2, 2048, 1024, 1024, 8
T = B * S            # 4096 tokens
NCORES = 8
P = 128
DK = D // P          # 8 contraction chunks over D
FK = F // P          # 8 F tiles
TCH = 384            # token chunk (3 PSUM banks of headroom per bank set)

_cache = {}


def _build_nc(C):
    """Expert kernel over C routed tokens (C a multiple of 128)."""
    import itertools
    from contextlib import ExitStack

    import concourse.mybir as mybir
    import concourse.tile as tile
    from concourse import bacc

    dt = mybir.dt
    AF = mybir.ActivationFunctionType
    ALU = mybir.AluOpType

    NTT = C // P
    chunks = []
    t = 0
    while t < C:
        c = min(TCH, C - t)
        chunks.append((t, c))
        t += c
    NCH = len(chunks)

    nc = bacc.Bacc("TRN2", target_bir_lowering=False, debug=False,
                   num_devices=NCORES)

    xg = nc.dram_tensor("xg", [D, C], dt.bfloat16, kind="ExternalInput").ap()
    w1t = nc.dram_tensor("w1t", [D, F], dt.bfloat16, kind="ExternalInput").ap()
    w3t = nc.dram_tensor("w3t", [D, F], dt.bfloat16, kind="ExternalInput").ap()
    w2t = nc.dram_tensor("w2t", [F, D], dt.bfloat16, kind="ExternalInput").ap()
    gs = nc.dram_tensor("gs", [P, NTT], dt.float32, kind="ExternalInput").ap()
    out = nc.dram_tensor("out", [C, D], dt.bfloat16, kind="ExternalOutput").ap()

    with tile.TileContext(nc) as tc, ExitStack() as ctx:
        const = ctx.enter_context(tc.tile_pool(name="const", bufs=1))
        spool = ctx.enter_context(tc.tile_pool(name="spool", bufs=2))
        hpool = ctx.enter_context(tc.tile_pool(name="hpool", bufs=2))
        ypool = ctx.enter_context(tc.tile_pool(name="ypool", bufs=3))
        psum = ctx.enter_context(tc.tile_pool(name="psum", bufs=1, space="PSUM"))

        # ---- input DMAs across the two HWDGE issue queues ----
        # DMA issues are flow-controlled by queue completion, so each
        # engine's stream delivers in issue order at the shared HBM rate.
        # scalar carries only the small early tiles (x chunk0 + gate
        # scales) so the critical w1 stream on sync gets the bandwidth;
        # sync then delivers in PE consumption order: w1, x chunk1, w3,
        # w2, x chunk2+.
        w1_sb = []
        xgc = [[None] * NCH for _ in range(DK)]
        for k in range(DK):
            t1 = const.tile([P, F], dt.bfloat16, tag=f"w1_{k}")
            nc.sync.dma_start(t1[:], w1t[k * P:(k + 1) * P, :])
            w1_sb.append(t1)
            tok, tch = chunks[0]
            tx = const.tile([P, tch], dt.bfloat16, tag=f"xg{k}c0")
            nc.scalar.dma_start(tx[:], xg[k * P:(k + 1) * P, tok:tok + tch])
            xgc[k][0] = tx
        gs_sb = const.tile([P, NTT], dt.float32, tag="gs")
        nc.scalar.dma_start(gs_sb[:], gs[:, :])
        if NCH >= 2:
            tok, tch = chunks[1]
            for k in range(DK):
                tx = const.tile([P, tch], dt.bfloat16, tag=f"xg{k}c1")
                nc.sync.dma_start(tx[:], xg[k * P:(k + 1) * P, tok:tok + tch])
                xgc[k][1] = tx
        w3_sb = []
        for k in range(DK):
            t3 = const.tile([P, F], dt.bfloat16, tag=f"w3_{k}")
            nc.sync.dma_start(t3[:], w3t[k * P:(k + 1) * P, :])
            w3_sb.append(t3)
        # w2 as one DMA into a [P, FK, D] tile; slice fk is w2t rows fk*128..
        w2_sb = const.tile([P, FK, D], dt.bfloat16, tag="w2")
        nc.sync.dma_start(
            w2_sb[:], w2t.rearrange("(k p) d -> p k d", p=P))
        for ci in range(2, NCH):
            tok, tch = chunks[ci]
            for k in range(DK):
                tx = const.tile([P, tch], dt.bfloat16, tag=f"xg{k}c{ci}")
                nc.sync.dma_start(tx[:], xg[k * P:(k + 1) * P, tok:tok + tch])
                xgc[k][ci] = tx

        slot = itertools.count()

        # PE warmup: a few matmuls on a zeroed dummy tile fill the otherwise
        # idle window while the first real tiles stream in, and trip the HAM
        # activity monitor early so real matmuls run at the full 2.4 GHz.
        dummy = const.tile([P, 512], dt.bfloat16, tag="dummy")
        nc.vector.memset(dummy[:], 0)
        psW = psum.tile([P, TCH], dt.float32, tag=f"b{next(slot) % 8}")
        for _ in range(12):
            nc.tensor.matmul(psW[:], lhsT=dummy[:, 0:P],
                             rhs=dummy[:, P:P + TCH], start=True, stop=True)

        psA = [None] * NCH
        psB = [None] * NCH
        ssb = [None] * NCH
        hsb = [None] * NCH

        def emit_A(ci):
            tok, tch = chunks[ci]
            tiles = []
            for f in range(FK):
                pa = psum.tile([P, tch], dt.float32, tag=f"b{next(slot) % 8}")
                tiles.append(pa)
            psA[ci] = tiles
            for k in range(DK):
                for f in range(FK):
                    nc.tensor.matmul(
                        tiles[f][:], lhsT=w1_sb[k][:, f * P:(f + 1) * P],
                        rhs=xgc[k][ci][:],
                        start=(k == 0), stop=(k == DK - 1))
            # silu into SBUF frees the psA banks for the next phase
            stiles = []
            for f in range(FK):
                st = spool.tile([P, tch], dt.float32, tag=f"s{f}")
                nc.scalar.activation(st[:], tiles[f][:], AF.Silu)
                stiles.append(st)
            ssb[ci] = stiles

        def emit_B(ci):
            tok, tch = chunks[ci]
            tiles = []
            for f in range(FK):
                pb = psum.tile([P, tch], dt.float32, tag=f"b{next(slot) % 8}")
                tiles.append(pb)
            psB[ci] = tiles
            for k in range(DK):
                for f in range(FK):
                    nc.tensor.matmul(
                        tiles[f][:], lhsT=w3_sb[k][:, f * P:(f + 1) * P],
                        rhs=xgc[k][ci][:],
                        start=(k == 0), stop=(k == DK - 1))
            htiles = []
            for f in range(FK):
                ht = hpool.tile([P, tch], dt.bfloat16, tag=f"h{f}")
                nc.vector.tensor_tensor(ht[:], ssb[ci][f][:], tiles[f][:],
                                        op=ALU.mult)
                htiles.append(ht)
            hsb[ci] = htiles

        def emit_G(ci):
            tok, tch = chunks[ci]
            for m in range(tch // P):
                jj = tok // P + m
                ysb = ypool.tile([P, D], dt.bfloat16, tag="ysb")
                psY0 = psum.tile([P, 512], dt.float32, tag=f"b{next(slot) % 8}")
                psY1 = psum.tile([P, 512], dt.float32, tag=f"b{next(slot) % 8}")
                for fk in range(FK):
                    hT = hsb[ci][fk][:, m * P:(m + 1) * P]
                    nc.tensor.matmul(
                        psY0[:], lhsT=hT,
                        rhs=w2_sb[:, fk, 0:512],
                        start=(fk == 0), stop=(fk == FK - 1))
                    nc.tensor.matmul(
                        psY1[:], lhsT=hT,
                        rhs=w2_sb[:, fk, 512:1024],
                        start=(fk == 0), stop=(fk == FK - 1))
                # scale+copy the two halves on different engines (ACT | DVE)
                # and issue their output DMAs from different queues, so the
                # per-tile epilogue runs in parallel rather than serially.
                nc.scalar.activation(ysb[:, 0:512], psY0[:], AF.Copy,
                                     scale=gs_sb[:, jj:jj + 1])
                nc.vector.tensor_scalar_mul(ysb[:, 512:1024], psY1[:],
                                            gs_sb[:, jj:jj + 1])
                nc.sync.dma_start(
                    out[tok + m * P: tok + (m + 1) * P, 0:512],
                    ysb[:, 0:512])
                nc.gpsimd.dma_start(
                    out[tok + m * P: tok + (m + 1) * P, 512:1024],
                    ysb[:, 512:1024])

        # software pipeline: A_{i+1} runs ahead so GEMM1-B never waits on w3
        if NCH >= 2:
            emit_A(0)
            emit_A(1)
        else:
            emit_A(0)
        for i in range(NCH):
            emit_B(i)
            emit_G(i)
            if i + 2 < NCH:
                emit_A(i + 2)

    nc.compile()
    return nc


def _route(x, gate_w):
    """Host gate: top-2 of 8, renormalized weights; per-expert token lists."""
    xf = np.ascontiguousarray(x.reshape(T, D).astype(np.float32))
    logits = xf @ gate_w.T.astype(np.float32)            # [T, E]
    order = np.argsort(-logits, axis=1, kind="stable")
    i1 = order[:, 0]
    i2 = order[:, 1]
    ar = np.arange(T)
    l1 = logits[ar, i1]
    l2 = logits[ar, i2]
    g1 = 1.0 / (1.0 + np.exp(l2 - l1))                   # renormalized top-2
    g2 = 1.0 - g1
    idxs, gws = [], []
    for e in range(E):
        m1 = i1 == e
        m2 = i2 == e
        idx = np.nonzero(m1 | m2)[0]
        g = np.where(m1, g1, g2)[idx].astype(np.float32)
        idxs.append(idx)
        gws.append(g)
    maxn = max(len(i) for i in idxs)
    C = max(512, -(-maxn // P) * P)
    return xf, idxs, gws, C


def _prepare(x, gate_w, w1, w3, w2):
    """Build (nc, in_maps, route_meta) for an SPMD run."""
    import ml_dtypes

    xf, idxs, gws, C = _route(x, gate_w)
    xTb = np.ascontiguousarray(xf.T).astype(ml_dtypes.bfloat16)  # [D, T]

    key = ("nc", C)
    if key not in _cache:
        _cache[key] = _build_nc(C)
    nc = _cache[key]

    in_maps = []
    for c in range(NCORES):
        idx = idxs[c]
        n = len(idx)
        xg = np.zeros((D, C), dtype=ml_dtypes.bfloat16)
        xg[:, :n] = xTb[:, idx]
        gpad = np.zeros(C, dtype=np.float32)
        gpad[:n] = gws[c]
        gs = np.ascontiguousarray(gpad.reshape(C // P, P).T)     # [P, NTT]
        in_maps.append({
            "xg": xg,
            "w1t": np.ascontiguousarray(w1[c].T).astype(ml_dtypes.bfloat16),
            "w3t": np.ascontiguousarray(w3[c].T).astype(ml_dtypes.bfloat16),
            "w2t": np.ascontiguousarray(w2[c].T).astype(ml_dtypes.bfloat16),
            "gs": gs,
        })
    return nc, in_maps, (idxs, C)


def _combine(results, meta):
    idxs, C = meta
    outf = np.zeros((T, D), dtype=np.float32)
    for e in range(E):
        idx = idxs[e]
        y = np.asarray(results[e]["out"])[:len(idx)].astype(np.float32)
        outf[idx] += y
    return outf.reshape(B, S, D)


def kernel(x, gate_w, w1, w3, w2):
    from concourse.bass_utils import run_bass_kernel_spmd

    nc, in_maps, meta = _prepare(x, gate_w, w1, w3, w2)
    res = run_bass_kernel_spmd(nc, in_maps, list(range(NCORES)))
    return _combine(res.results, meta)
